# revision 43
# baseline (speedup 1.0000x reference)
"""Trainium2 Bass kernel for nn_CHAN_29764123361280 (ragged_sequence).

Sharding: data-parallel over the batch axis B=8 -> one video per NeuronCore,
all weights replicated. Per-core pipeline (32 segments):
  conv1(k5,p2)+maxpool2 -> conv2(k5,p2)+maxpool2 -> additive self-attention
  + two concept attentions -> concat -> deconv1 -> deconv2 -> similarity
  scoring.  All matmuls bf16 with fp32 PSUM accumulation.

Per-segment additive-attention scores are packed into PSUM rows with a
"banded v" trick: lhsT is a [128,8] slice of a banded matrix whose only
nonzero column is sa_v placed so that segment-local row `sub` receives
v . tanh(qp+kp), accumulated over both 128-chunks of the hidden dim.
"""

from contextlib import ExitStack

import numpy as np
import ml_dtypes

import concourse.bass as bass
import concourse.mybir as mybir
import concourse.tile as tile
from concourse import bacc
from concourse.bass_utils import run_bass_kernel_spmd
from concourse.masks import make_identity

BF16 = mybir.dt.bfloat16
F32 = mybir.dt.float32

B, S, L, CIN = 8, 32, 128, 1024
C1, C2 = 512, 256
D1, D2 = 512, 128
CD, SIM = 300, 128
L4 = L // 4          # 32
SEG = S              # segments per core
NEG = -30.0          # mask logit bias (exp(-30) ~ 1e-13)

nbf = ml_dtypes.bfloat16

DC_TAPS = ((0, ((1, 1), (3, 0))), (1, ((2, 1), (0, 2))))
# parity -> ((tap, input col offset), ...) for ConvTranspose1d(k=4,s=2,p=1)
# on halo'd input xh[:, u+1] = x[:, u]:
#   even out j=2u:  W1.x[u]  + W3.x[u-1]
#   odd  out j=2u+1: W2.x[u] + W0.x[u+1]


def _emit(ctx, tc, io):
    nc = tc.nc
    AX = mybir.AxisListType
    OP = mybir.AluOpType
    ACT = mybir.ActivationFunctionType

    singles = ctx.enter_context(tc.tile_pool(name="singles", bufs=1))
    # ---- resident weights / constants ----
    c1w = singles.tile([128, 8, 5, C1], BF16)
    c2w = singles.tile([128, 4, 5, C2], BF16)
    dc1w = singles.tile([128, 8, 4, D1], BF16)
    dc2w = singles.tile([128, 4, 4, D2], BF16)
    saq = singles.tile([128, 2, C2], BF16)
    sak = singles.tile([128, 2, C2], BF16)
    cak = singles.tile([128, 2, C2], BF16)
    caq = singles.tile([128, 3, C2], BF16)
    s1w = singles.tile([128, SIM], BF16)
    s2w = singles.tile([128, 3, SIM], BF16)
    vband = singles.tile([128, 2, 256], BF16)
    cavband = singles.tile([128, 2, 256], BF16)
    c1b = singles.tile([128, 4], F32)
    c2b = singles.tile([128, 2], F32)
    dc1b = singles.tile([128, 4], F32)
    dc2b = singles.tile([128, 1], F32)
    sbqk = singles.tile([128, 2], F32)
    cab = singles.tile([128, 2], F32)
    mlpw = singles.tile([128, 1], F32)
    mlpb = singles.tile([16, 1], F32)
    cvec = singles.tile([128, 3, 2], BF16)
    mb = singles.tile([8, 4, L4], F32)     # [local row, supergroup, k]
    ident = singles.tile([128, 128], BF16)

    for t_sb, name in [
        (c1w, "c1w"), (c2w, "c2w"), (dc1w, "dc1w"), (dc2w, "dc2w"),
        (saq, "saq"), (sak, "sak"), (cak, "cak"), (caq, "caq"),
        (s1w, "s1w"), (s2w, "s2w"), (vband, "vband"), (cavband, "cavband"),
        (c1b, "c1b"), (c2b, "c2b"), (dc1b, "dc1b"), (dc2b, "dc2b"),
        (sbqk, "sbqk"), (cab, "cab"), (mlpw, "mlpw"),
        (mlpb, "mlpb"), (cvec, "cvec"), (mb, "mb"),
    ]:
        nc.sync.dma_start(out=t_sb[:], in_=io[name])
    make_identity(nc, ident[:])

    # Touch every DMA'd tensor an engine will read, one instruction per
    # tensor, so each engine's vector clock observes the DMA semaphores
    # early: later compute then never needs >1 sync wait per instruction
    # (the walrus TT/STT encodings only carry one).
    dve_scr = singles.tile([1, 16], F32)
    act_scr = singles.tile([1, 16], F32)
    for i, t_sb in enumerate(
            (cab, mlpw, mb[:, 0, :], dc1b, dc2b, sbqk, c1b, c2b, mlpb)):
        nc.vector.tensor_copy(out=dve_scr[0:1, i : i + 1], in_=t_sb[0:1, 0:1])
        nc.scalar.copy(out=act_scr[0:1, i : i + 1], in_=t_sb[0:1, 0:1])

    # ---- pools ----
    xp = ctx.enter_context(tc.tile_pool(name="xp", bufs=2))
    t1p = ctx.enter_context(tc.tile_pool(name="t1p", bufs=2))
    t2p = ctx.enter_context(tc.tile_pool(name="t2p", bufs=2))
    atp = ctx.enter_context(tc.tile_pool(name="atp", bufs=2))
    bbp = ctx.enter_context(tc.tile_pool(name="bbp", bufs=2))
    arp = ctx.enter_context(tc.tile_pool(name="arp", bufs=2))
    r1p = ctx.enter_context(tc.tile_pool(name="r1p", bufs=2))
    r2p = ctx.enter_context(tc.tile_pool(name="r2p", bufs=2))
    smp = ctx.enter_context(tc.tile_pool(name="smp", bufs=2))
    php = ctx.enter_context(tc.tile_pool(name="php", bufs=1))
    finp = ctx.enter_context(tc.tile_pool(name="finp", bufs=2))

    wps = ctx.enter_context(tc.tile_pool(name="wps", bufs=2, space="PSUM"))
    pps = ctx.enter_context(tc.tile_pool(name="pps", bufs=1, space="PSUM"))
    sps = ctx.enter_context(tc.tile_pool(name="sps", bufs=1, space="PSUM"))
    scp = ctx.enter_context(tc.tile_pool(name="scp", bufs=1, space="PSUM"))

    # ---- phase 0: concept-dependent vectors ----
    cqb = php.tile([128, 2, 2], F32)          # tanh bias for concept attn
    uband = php.tile([128, 512], BF16)        # banded (ci-interleaved) u
    cq_ps = wps.tile([128, 2, 2], F32, tag="work")
    for i, (ci, ht) in enumerate([(c, h) for c in range(2) for h in range(2)]):
        for kc in range(3):
            nc.tensor.matmul(
                cq_ps[:, ht, ci : ci + 1],
                caq[:, kc, ht * 128 : (ht + 1) * 128],
                cvec[:, kc, ci : ci + 1],
                start=(i == 0 and kc == 0), stop=(i == 3 and kc == 2))
    for ci in range(2):
        for ht in range(2):
            nc.vector.tensor_tensor(
                out=cqb[:, ht, ci : ci + 1], in0=cq_ps[:, ht, ci : ci + 1],
                in1=cab[:, ht : ht + 1], op=OP.add)
    cw_ps = wps.tile([128, 2], F32, tag="work")
    for ci in range(2):
        for kc in range(3):
            nc.tensor.matmul(
                cw_ps[:, ci : ci + 1], s2w[:, kc, :], cvec[:, kc, ci : ci + 1],
                start=(ci == 0 and kc == 0), stop=(ci == 1 and kc == 2))
    nc.vector.memset(uband[:], 0.0)
    for ci in range(2):
        nc.vector.tensor_tensor(
            out=uband[:, 256 + ci : 257 + ci], in0=cw_ps[:, ci : ci + 1],
            in1=mlpw[:, 0:1], op=OP.mult)
    nc.scalar.copy(out=act_scr[0:1, 15:16], in_=cqb[0:1, 0, 0:1])
    tc.no_sync_barrier()

    for sg in range(4):                       # supergroups of 8 segments
        tmp2 = t2p.tile([128, 2, 8, L4], BF16, tag="tmp2")
        # ============ conv stack, in groups of 4 segments ============
        for g in range(2):
            xg = xp.tile([128, 8, 4, 132], BF16, tag="xg")
            src = io["x"][sg * 8 + g * 4 : sg * 8 + g * 4 + 4].transpose(
                [1, 2, 0, 3])
            nc.sync.dma_start(out=xg[:], in_=src)

            t1 = t1p.tile([128, 4, 4, 68], BF16, tag="t1")
            nc.vector.memset(t1[:, :, :, 0:2], 0.0)
            nc.vector.memset(t1[:, :, :, 66:68], 0.0)
            for m in range(4):
                y1 = wps.tile([128, 4, 128], F32, tag="work")
                n_mm = 0
                for kc in range(8):
                    for t in range(5):
                        nc.tensor.matmul(
                            y1[:], c1w[:, kc, t, m * 128 : (m + 1) * 128],
                            xg[:, kc, :, t : t + 128],
                            start=(n_mm == 0), stop=(n_mm == 39))
                        n_mm += 1
                ys = t1p.tile([128, 4, 128], F32, tag="pool1")
                nc.scalar.activation(out=ys[:], in_=y1[:], func=ACT.Identity,
                                     bias=c1b[:, m : m + 1])
                yv = ys[:].rearrange("p s (u two) -> p s u two", two=2)
                nc.vector.tensor_tensor(
                    out=t1[:, m, :, 2:66], in0=yv[:, :, :, 0],
                    in1=yv[:, :, :, 1], op=OP.max)

            for m in range(2):
                y2 = wps.tile([128, 4, 64], F32, tag="work")
                n_mm = 0
                for kc in range(4):
                    for t in range(5):
                        nc.tensor.matmul(
                            y2[:], c2w[:, kc, t, m * 128 : (m + 1) * 128],
                            t1[:, kc, :, t : t + 64],
                            start=(n_mm == 0), stop=(n_mm == 19))
                        n_mm += 1
                ys = t1p.tile([128, 4, 64], F32, tag="pool2")
                nc.scalar.activation(out=ys[:], in_=y2[:], func=ACT.Identity,
                                     bias=c2b[:, m : m + 1])
                yv = ys[:].rearrange("p s (u two) -> p s u two", two=2)
                nc.vector.tensor_tensor(
                    out=tmp2[:, m, g * 4 : g * 4 + 4, :], in0=yv[:, :, :, 0],
                    in1=yv[:, :, :, 1], op=OP.max)

        # ===== tmp2 transposed: [qh*32+k, seg, c], 2x replicated over qh ====
        t2kc = t2p.tile([32, 8, C2], BF16, tag="t2kc")
        for sub in range(8):
            for m in range(2):
                tp = wps.tile([32, 128], BF16, tag="work")
                nc.tensor.transpose(tp[:], tmp2[:, m, sub, :], ident[:])
                nc.vector.tensor_copy(
                    out=t2kc[:, sub, m * 128 : (m + 1) * 128], in_=tp[:])

        # ============ attention projections ============
        qk_sb = atp.tile([128, 2, 2, 8, L4], F32, tag="qk")   # [q/k, ht, ...]
        tcn = atp.tile([128, 2, 2, 8, L4], BF16, tag="tcn")   # [ht, ci, ...]
        for ht in range(2):
            qkc = pps.tile([128, 2, 512], F32, tag="qkc")  # bank per target
            for tgt, w_sb in enumerate((saq, sak)):
                tv = qkc[:, tgt, 0:256].rearrange("p (s k) -> p s k", k=L4)
                for kc in range(2):
                    nc.tensor.matmul(
                        tv, w_sb[:, kc, ht * 128 : (ht + 1) * 128],
                        tmp2[:, kc, :, :], start=(kc == 0), stop=(kc == 1))
            nc.vector.tensor_scalar_add(
                out=qk_sb[:, 0, ht, :, :],
                in0=qkc[:, 0, 0:256].rearrange("p (s k) -> p s k", k=L4),
                scalar1=sbqk[:, ht : ht + 1])
            nc.vector.tensor_copy(
                out=qk_sb[:, 1, ht, :, :],
                in_=qkc[:, 1, 0:256].rearrange("p (s k) -> p s k", k=L4))
            ck = wps.tile([128, 8, L4], F32, tag="work")
            for kc in range(2):
                nc.tensor.matmul(
                    ck[:], cak[:, kc, ht * 128 : (ht + 1) * 128],
                    tmp2[:, kc, :, :], start=(kc == 0), stop=(kc == 1))
            for ci in range(2):
                nc.scalar.activation(
                    out=tcn[:, ht, ci, :, :], in_=ck[:],
                    func=ACT.Tanh, bias=cqb[:, ht, ci : ci + 1])

        # ====== additive scores, scattered to local psum rows 0..7 ======
        s_ps = sps.tile([8, 2, 512], F32, tag="s")        # [sub, half, qk]
        sc_ps = sps.tile([8, 2, L4], F32, tag="sc")       # [sub, ci, k]
        for sub in range(8):
            tts = bbp.tile([128, 2, 1024], BF16, tag="tts")
            for ht in range(2):
                bb = bbp.tile([128, L4, L4], F32, tag="bb")
                nc.vector.tensor_tensor(
                    out=bb[:],
                    in0=qk_sb[:, 0, ht, sub, :].unsqueeze(2).broadcast_to(
                        [128, L4, L4]),
                    in1=qk_sb[:, 1, ht, sub, :].unsqueeze(1).broadcast_to(
                        [128, L4, L4]),
                    op=OP.add)
                nc.scalar.activation(
                    out=tts[:, ht, :].rearrange("p (q k) -> p q k", k=L4),
                    in_=bb[:], func=ACT.Tanh)
            for half in range(2):
                for ht in range(2):
                    nc.tensor.matmul(
                        s_ps[:, half, :],
                        vband[:, ht, 128 - sub : 136 - sub],
                        tts[:, ht, half * 512 : (half + 1) * 512],
                        start=(ht == 0 and sub == 0),
                        stop=(ht == 1 and sub == 7))
            for ht in range(2):
                nc.tensor.matmul(
                    sc_ps[:, :, :],
                    cavband[:, ht, 128 - sub : 136 - sub],
                    tcn[:, ht, :, sub, :],
                    start=(ht == 0 and sub == 0),
                    stop=(ht == 1 and sub == 7))

        # ============ masked softmaxes (rows 0..7) ============
        sv = s_ps[:].rearrange("s h (q k) -> s (h q) k", k=L4)
        nc.vector.tensor_tensor(
            out=sv[:], in0=sv[:],
            in1=mb[:, sg, :].unsqueeze(1).broadcast_to([8, L4, L4]),
            op=OP.add)
        nc.scalar.activation(out=sv[:], in_=sv[:], func=ACT.Exp)
        zs = smp.tile([8, L4], F32, tag="zs")
        nc.vector.reduce_sum(out=zs[:], in_=sv[:], axis=AX.X)
        nc.vector.reciprocal(out=zs[:], in_=zs[:])
        a_sb = smp.tile([8, L4, L4], BF16, tag="a_sb")
        nc.vector.tensor_tensor(
            out=a_sb[:], in0=sv[:],
            in1=zs[:].unsqueeze(2).broadcast_to([8, L4, L4]), op=OP.mult)

        for ci in range(2):
            nc.vector.tensor_tensor(
                out=sc_ps[:, ci, :], in0=sc_ps[:, ci, :],
                in1=mb[:, sg, :], op=OP.add)
        nc.scalar.activation(out=sc_ps[:], in_=sc_ps[:], func=ACT.Exp)
        zc = smp.tile([8, 2], F32, tag="zc")
        nc.vector.reduce_sum(out=zc[:], in_=sc_ps[:], axis=AX.X)
        nc.vector.reciprocal(out=zc[:], in_=zc[:])
        ac_sb = smp.tile([8, 2, L4], BF16, tag="ac_sb")
        nc.vector.tensor_tensor(
            out=ac_sb[:], in0=sc_ps[:],
            in1=zc[:].unsqueeze(2).broadcast_to([8, 2, L4]), op=OP.mult)

        # ====== transpose attention weights: k onto partitions ======
        aT_ps = wps.tile([32, L4, 8], BF16, tag="work")
        for q in range(L4):
            nc.tensor.matmul(
                aT_ps[:, q, :], a_sb[:, q, :],
                ident[0:8, 0:8], is_transpose=True,
                start=(q == 0), stop=(q == L4 - 1))
        aT = atp.tile([32, L4, 8], BF16, tag="aTs")   # [k, q, sub]
        nc.vector.tensor_copy(out=aT[:], in_=aT_ps[:])
        acT_ps = wps.tile([32, 2, 8], BF16, tag="work")
        for ci in range(2):
            nc.tensor.matmul(
                acT_ps[:, ci, :], ac_sb[:, ci, :], ident[0:8, 0:8],
                is_transpose=True, start=(ci == 0), stop=(ci == 1))
        acT = atp.tile([32, 2, 8], BF16, tag="acTs")  # [k, ci, sub]
        nc.vector.tensor_copy(out=acT[:], in_=acT_ps[:])

        # ====== per-segment: self_res, concept_res, concat ======
        ar = arp.tile([128, 8, 8, 34], BF16, tag="ar")
        nc.vector.memset(ar[:, :, :, 0:1], 0.0)
        nc.vector.memset(ar[:, :, :, 33:34], 0.0)
        for m in range(2):
            nc.vector.tensor_copy(out=ar[:, m, :, 1:33], in_=tmp2[:, m, :, :])
        for sub in range(8):
            srp = wps.tile([128, 2, 34], F32, tag="work")
            n_mm = 0
            for m in range(2):
                for qh in range(2):
                    nc.tensor.matmul(
                        srp[:, m, qh * 16 : (qh + 1) * 16],
                        t2kc[:, sub, m * 128 : (m + 1) * 128],
                        aT[:, qh * 16 : (qh + 1) * 16, sub],
                        start=(n_mm == 0), stop=(n_mm == 7))
                    n_mm += 1
                for ci in range(2):
                    nc.tensor.matmul(
                        srp[:, m, 32 + ci : 33 + ci],
                        t2kc[0:32, sub, m * 128 : (m + 1) * 128],
                        acT[0:32, ci, sub : sub + 1],
                        start=(n_mm == 0), stop=(n_mm == 7))
                    n_mm += 1
            nc.vector.tensor_copy(out=ar[:, 2:4, sub, 1:33],
                                  in_=srp[:, :, 0:32])
            for ci in range(2):
                for m in range(2):
                    nc.vector.tensor_copy(
                        out=ar[:, 4 + 2 * ci + m, sub, 1:33],
                        in_=srp[:, m, 32 + ci : 33 + ci].broadcast_to(
                            [128, L4]))

        # ============ deconv1: [1024,32] -> [512,64] ============
        r1t = r1p.tile([128, 4, 8, 66], BF16, tag="r1t")
        nc.vector.memset(r1t[:, :, :, 0:1], 0.0)
        nc.vector.memset(r1t[:, :, :, 65:66], 0.0)
        for m in range(4):
            for par, taps in DC_TAPS:
                d1 = wps.tile([128, 8, L4], F32, tag="work")
                n_mm = 0
                for kc in range(8):
                    for t, off in taps:
                        nc.tensor.matmul(
                            d1[:], dc1w[:, kc, t, m * 128 : (m + 1) * 128],
                            ar[:, kc, :, off : off + 32],
                            start=(n_mm == 0), stop=(n_mm == 15))
                        n_mm += 1
                nc.vector.tensor_scalar_add(
                    out=r1t[:, m, :, 1 + par : 65 + par : 2], in0=d1[:],
                    scalar1=dc1b[:, m : m + 1])

        # ============ deconv2: [512,64] -> [128,128] ============
        r2t = r2p.tile([128, 8, 128], BF16, tag="r2t")
        for par, taps in DC_TAPS:
            d2 = wps.tile([128, 8, 64], F32, tag="work")
            n_mm = 0
            for kc in range(4):
                for t, off in taps:
                    nc.tensor.matmul(
                        d2[:], dc2w[:, kc, t, :],
                        r1t[:, kc, :, off : off + 64],
                        start=(n_mm == 0), stop=(n_mm == 7))
                    n_mm += 1
            nc.vector.tensor_scalar_add(
                out=r2t[:, :, par : 128 : 2], in0=d2[:], scalar1=dc2b[:, 0:1])

        # ============ scoring ============
        score_ps = scp.tile([16, 128], F32, tag="score")   # [(ci,sub), l]
        for sub in range(8):
            sim_ps = wps.tile([128, 128], F32, tag="work")
            nc.tensor.matmul(sim_ps[:], s1w[:], r2t[:, sub, :],
                             start=True, stop=True)
            sim_sb = smp.tile([128, 128], BF16, tag="sim_sb")
            nc.vector.tensor_copy(out=sim_sb[:], in_=sim_ps[:])
            nc.tensor.matmul(
                score_ps[:],
                uband[:, (128 - sub) * 2 : (128 - sub) * 2 + 16], sim_sb[:],
                start=(sub == 0), stop=(sub == 7))
        final = finp.tile([16, 128], F32, tag="final")
        nc.scalar.activation(out=final[:], in_=score_ps[:], func=ACT.Sigmoid,
                             bias=mlpb[:, 0:1])
        nc.sync.dma_start(
            out=io["out"].transpose([1, 0, 2])[sg * 8 : sg * 8 + 8],
            in_=final[:])


# ---------------------------------------------------------------------------
# program build (cached)
# ---------------------------------------------------------------------------

_CACHE = {}


def _build():
    if "nc" in _CACHE:
        return _CACHE["nc"]
    nc = bacc.Bacc("TRN2", target_bir_lowering=False, debug=False)
    d = {}

    def di(name, shape, dt):
        d[name] = nc.dram_tensor(name, shape, dt, kind="ExternalInput").ap()

    di("x", [SEG, 128, 8, 132], BF16)
    di("c1w", [128, 8, 5, C1], BF16)
    di("c2w", [128, 4, 5, C2], BF16)
    di("dc1w", [128, 8, 4, D1], BF16)
    di("dc2w", [128, 4, 4, D2], BF16)
    di("saq", [128, 2, C2], BF16)
    di("sak", [128, 2, C2], BF16)
    di("cak", [128, 2, C2], BF16)
    di("caq", [128, 3, C2], BF16)
    di("s1w", [128, SIM], BF16)
    di("s2w", [128, 3, SIM], BF16)
    di("vband", [128, 2, 256], BF16)
    di("cavband", [128, 2, 256], BF16)
    di("c1b", [128, 4], F32)
    di("c2b", [128, 2], F32)
    di("dc1b", [128, 4], F32)
    di("dc2b", [128, 1], F32)
    di("sbqk", [128, 2], F32)
    di("cab", [128, 2], F32)
    di("mlpw", [128, 1], F32)
    di("mlpb", [16, 1], F32)
    di("cvec", [128, 3, 2], BF16)
    di("mb", [8, 4, L4], F32)
    d["out"] = nc.dram_tensor("out", [2, SEG, 128], F32,
                              kind="ExternalOutput").ap()
    with tile.TileContext(nc) as tc:
        with ExitStack() as ctx:
            _emit(ctx, tc, d)
    nc.compile()
    _CACHE["nc"] = nc
    return nc


# ---------------------------------------------------------------------------
# host-side prep (layout/cast only)
# ---------------------------------------------------------------------------

def _chunk_bias(v, nchunk):
    return np.ascontiguousarray(
        np.asarray(v, np.float32).reshape(nchunk, 128).T)


def _band(v):
    # [128, 2, 256] bf16; column 128 of chunk ht = v[ht*128:(ht+1)*128]
    out = np.zeros((128, 2, 256), nbf)
    vv = np.asarray(v, np.float32).reshape(2, 128).T
    out[:, :, 128] = vv.astype(nbf)
    return out


def _wchunks(w, nk):
    # w: [K, ...] -> [128, nk, ...] (zero-pad K up to nk*128)
    w = np.asarray(w, np.float32)
    k = w.shape[0]
    if k < nk * 128:
        w = np.concatenate(
            [w, np.zeros((nk * 128 - k,) + w.shape[1:], np.float32)], 0)
    w = w.reshape((nk, 128) + w.shape[1:])
    perm = (1, 0) + tuple(range(2, w.ndim))
    return np.ascontiguousarray(w.transpose(perm)).astype(nbf)


def prepare_common(inp):
    g = {}
    g["c1w"] = _wchunks(np.asarray(inp["conv1_w"], np.float32)
                        .transpose(1, 2, 0), 8)       # [128,8,5,512]
    g["c2w"] = _wchunks(np.asarray(inp["conv2_w"], np.float32)
                        .transpose(1, 2, 0), 4)       # [128,4,5,256]
    g["dc1w"] = _wchunks(np.asarray(inp["dc1_w"], np.float32)
                         .transpose(0, 2, 1), 8)      # [128,8,4,512]
    g["dc2w"] = _wchunks(np.asarray(inp["dc2_w"], np.float32)
                         .transpose(0, 2, 1), 4)      # [128,4,4,128]
    g["saq"] = _wchunks(inp["sa_wq"], 2)
    g["sak"] = _wchunks(inp["sa_wk"], 2)
    g["cak"] = _wchunks(inp["ca_wk"], 2)
    g["caq"] = _wchunks(inp["ca_wq"], 3)              # [128,3,256]
    g["s1w"] = np.ascontiguousarray(
        np.asarray(inp["sim1_w"], np.float32)).astype(nbf)
    g["s2w"] = _wchunks(inp["sim2_w"], 3)             # [128,3,128]
    g["vband"] = _band(inp["sa_v"])
    g["cavband"] = _band(inp["ca_v"])
    g["c1b"] = _chunk_bias(inp["conv1_b"], 4)
    g["c2b"] = _chunk_bias(inp["conv2_b"], 2)
    g["dc1b"] = _chunk_bias(inp["dc1_b"], 4)
    g["dc2b"] = _chunk_bias(inp["dc2_b"], 1)
    g["sbqk"] = _chunk_bias(
        np.asarray(inp["sa_bq"], np.float32)
        + np.asarray(inp["sa_bk"], np.float32), 2)
    g["cab"] = _chunk_bias(
        np.asarray(inp["ca_bq"], np.float32)
        + np.asarray(inp["ca_bk"], np.float32), 2)
    g["mlpw"] = np.ascontiguousarray(
        np.asarray(inp["mlp_w"], np.float32).reshape(128, 1))
    g["mlpb"] = np.full((16, 1), float(np.asarray(inp["mlp_b"])), np.float32)
    return g


def prepare_core(inp, b):
    o = {}
    x = np.asarray(inp["batch"], np.float32)[b]       # [32,128,1024]
    x = x.transpose(0, 2, 1)                          # [32,1024,128]
    xp = np.zeros((SEG, CIN, 132), np.float32)
    xp[:, :, 2:130] = x
    xp = xp.reshape(SEG, 8, 128, 132).transpose(0, 2, 1, 3)
    o["x"] = np.ascontiguousarray(xp).astype(nbf)     # [32,128,8,132]
    cv = np.zeros((2, 384), np.float32)
    cv[0, :CD] = np.asarray(inp["concept1"], np.float32)[b]
    cv[1, :CD] = np.asarray(inp["concept2"], np.float32)[b]
    o["cvec"] = np.ascontiguousarray(
        cv.reshape(2, 3, 128).transpose(2, 1, 0)).astype(nbf)  # [128,3,2]
    sl = np.asarray(inp["seg_len"], np.int64)[b]      # [32]
    k = np.arange(L4)
    m = np.where(sl[:, None] > 4 * k[None, :], 0.0, NEG).astype(np.float32)
    o["mb"] = np.ascontiguousarray(
        m.reshape(4, 8, L4).transpose(1, 0, 2))       # [8, 4, 32]
    return o


def kernel(**inputs):
    nc = _build()
    common = prepare_common(inputs)
    in_maps = []
    for b in range(B):
        m = dict(common)
        m.update(prepare_core(inputs, b))
        in_maps.append(m)
    res = run_bass_kernel_spmd(nc, in_maps, list(range(B)))
    s1 = np.stack([res.results[b]["out"][0] for b in range(B)])
    s2 = np.stack([res.results[b]["out"][1] for b in range(B)])
    return s1.astype(np.float32), s2.astype(np.float32)


# revision 51
# speedup vs baseline: 148.5987x; 148.5987x over previous
"""Trainium2 Bass kernel for nn_CHAN_29764123361280 (ragged_sequence).

Sharding: data-parallel over the batch axis B=8 -> one video per NeuronCore,
all weights replicated. Per-core pipeline (32 segments):
  conv1(k5,p2)+maxpool2 -> conv2(k5,p2)+maxpool2 -> additive self-attention
  + two concept attentions -> concat -> deconv1 -> deconv2 -> similarity
  scoring.  All matmuls bf16 with fp32 PSUM accumulation.

Per-segment additive-attention scores are packed into PSUM rows with a
"banded v" trick: lhsT is a [128,8] slice of a banded matrix whose only
nonzero column is sa_v placed so that segment-local row `sub` receives
v . tanh(qp+kp), accumulated over both 128-chunks of the hidden dim.

The conv stage for supergroup sg+1 is emitted between the projection and
attention phases of supergroup sg so the PE has conv work while the
DVE/ACT tanh chain runs.
"""

from contextlib import ExitStack

import numpy as np
import ml_dtypes

import concourse.bass as bass  # noqa: F401
import concourse.mybir as mybir
import concourse.tile as tile
from concourse import bacc
from concourse.bass_utils import run_bass_kernel_spmd
from concourse.masks import make_identity

BF16 = mybir.dt.bfloat16
F32 = mybir.dt.float32

B, S, L, CIN = 8, 32, 128, 1024
C1, C2 = 512, 256
D1, D2 = 512, 128
CD, SIM = 300, 128
L4 = L // 4          # 32
SEG = S              # segments per core
NEG = -30.0          # mask logit bias (exp(-30) ~ 1e-13)

nbf = ml_dtypes.bfloat16

DC_TAPS = ((0, ((1, 1), (3, 0))), (1, ((2, 1), (0, 2))))
# parity -> ((tap, input col offset), ...) for ConvTranspose1d(k=4,s=2,p=1)
# on halo'd input xh[:, u+1] = x[:, u]:
#   even out j=2u:  W1.x[u]  + W3.x[u-1]
#   odd  out j=2u+1: W2.x[u] + W0.x[u+1]

AX = mybir.AxisListType
OP = mybir.AluOpType
ACTF = mybir.ActivationFunctionType


class _Env:
    pass


def _setup(ctx, tc, io):
    nc = tc.nc
    e = _Env()
    e.nc, e.tc, e.io = nc, tc, io
    singles = ctx.enter_context(tc.tile_pool(name="singles", bufs=1))
    e.singles = singles
    # ---- resident weights / constants ----
    e.c1wk = [singles.tile([128, 5, C1], BF16, name=f"c1w{i}",
                           tag=f"c1w{i}") for i in range(8)]
    e.c2w = singles.tile([128, 4, 5, C2], BF16)
    e.dc1w = singles.tile([128, 8, 4, D1], BF16)
    e.dc2w = singles.tile([128, 4, 4, D2], BF16)
    e.saq = singles.tile([128, 2, C2], BF16)
    e.sak = singles.tile([128, 2, C2], BF16)
    e.cak = singles.tile([128, 2, C2], BF16)
    e.caq = singles.tile([128, 3, C2], BF16)
    e.s1w = singles.tile([128, SIM], BF16)
    e.s2w = singles.tile([128, 3, SIM], BF16)
    e.vband = singles.tile([128, 2, 256], BF16)
    e.cavband = singles.tile([128, 2, 256], BF16)
    e.c1b = singles.tile([128, 4], F32)
    e.c2b = singles.tile([128, 2], F32)
    e.dc1b = singles.tile([128, 4], F32)
    e.dc2b = singles.tile([128, 1], F32)
    e.sbqk = singles.tile([128, 2], F32)
    e.cab = singles.tile([128, 2], F32)
    e.mlpw = singles.tile([128, 1], F32)
    e.mlpb = singles.tile([16, 1], F32)
    e.cvec = singles.tile([128, 3, 2], BF16)
    e.mb = singles.tile([8, 4, L4], F32)    # [local row, supergroup, k]
    e.ident = singles.tile([128, 128], BF16)

    # ---- pools ----
    e.xp = ctx.enter_context(tc.tile_pool(name="xp", bufs=3))
    e.t1p = ctx.enter_context(tc.tile_pool(name="t1p", bufs=2))
    e.t2p = ctx.enter_context(tc.tile_pool(name="t2p", bufs=2))
    e.atp = ctx.enter_context(tc.tile_pool(name="atp", bufs=2))
    e.bbp = ctx.enter_context(tc.tile_pool(name="bbp", bufs=3))
    e.arp = ctx.enter_context(tc.tile_pool(name="arp", bufs=2))
    e.r1p = ctx.enter_context(tc.tile_pool(name="r1p", bufs=2))
    e.r2p = ctx.enter_context(tc.tile_pool(name="r2p", bufs=2))
    e.smp = ctx.enter_context(tc.tile_pool(name="smp", bufs=2))
    e.php = ctx.enter_context(tc.tile_pool(name="php", bufs=1))
    e.finp = ctx.enter_context(tc.tile_pool(name="finp", bufs=2))

    # PSUM: conv(2x1) + work(2x1) + s(2) + sc(1) + score(1) = 8 banks
    e.wps = ctx.enter_context(tc.tile_pool(name="wps", bufs=2, space="PSUM"))
    e.sps = ctx.enter_context(tc.tile_pool(name="sps", bufs=1, space="PSUM"))
    e.scp = ctx.enter_context(tc.tile_pool(name="scp", bufs=1, space="PSUM"))

    e.xg0 = []
    for g in range(2):
        xg = e.xp.tile([128, 8, 4, 132], BF16, name=f"xg0{g}", tag="xg")
        nc.sync.dma_start(out=xg[:], in_=io["x"][g])
        e.xg0.append(xg)
    for t_sb, name in [
        (e.cvec, "cvec"), (e.caq, "caq"), (e.s2w, "s2w"),
        (e.c1b, "c1b"), (e.c2b, "c2b"), (e.dc1b, "dc1b"), (e.dc2b, "dc2b"),
        (e.sbqk, "sbqk"), (e.cab, "cab"), (e.mlpw, "mlpw"),
        (e.mlpb, "mlpb"), (e.mb, "mb"),
        (e.vband, "vband"), (e.cavband, "cavband"),
    ]:
        nc.sync.dma_start(out=t_sb[:], in_=io[name])
    for i in range(8):
        nc.sync.dma_start(out=e.c1wk[i][:], in_=io["c1w"][:, i])
    nc.sync.dma_start(out=e.c2w[:], in_=io["c2w"])
    nc.sync.dma_start(out=e.saq[:], in_=io["saq"])
    nc.sync.dma_start(out=e.sak[:], in_=io["sak"])
    nc.sync.dma_start(out=e.cak[:], in_=io["cak"])
    make_identity(nc, e.ident[:])

    # Touch every DMA'd tensor an engine will read, one instruction per
    # tensor, so each engine's vector clock observes the DMA semaphores
    # early: later compute then never needs >1 sync wait per instruction
    # (the walrus TT/STT encodings only carry one).
    e.dve_scr = singles.tile([1, 16], F32)
    e.act_scr = singles.tile([1, 16], F32)
    for i, t_sb in enumerate(
            (e.cab, e.mlpw, e.mb[:, 0, :], e.dc1b, e.dc2b, e.sbqk,
             e.c1b, e.c2b, e.mlpb)):
        nc.vector.tensor_copy(out=e.dve_scr[0:1, i : i + 1],
                              in_=t_sb[0:1, 0:1])
        nc.scalar.copy(out=e.act_scr[0:1, i : i + 1], in_=t_sb[0:1, 0:1])

    return e


def _phase0(e):
    nc = e.nc
    e.cqb = e.php.tile([128, 2, 2], F32)       # tanh bias for concept attn
    e.uband = e.php.tile([128, 512], BF16)     # banded (ci-interleaved) u
    cq_ps = e.wps.tile([128, 2, 2], F32, tag="work")
    for i, (ci, ht) in enumerate([(c, h) for c in range(2) for h in range(2)]):
        for kc in range(3):
            nc.tensor.matmul(
                cq_ps[:, ht, ci : ci + 1],
                e.caq[:, kc, ht * 128 : (ht + 1) * 128],
                e.cvec[:, kc, ci : ci + 1],
                start=(i == 0 and kc == 0), stop=(i == 3 and kc == 2))
    for ci in range(2):
        for ht in range(2):
            nc.vector.tensor_tensor(
                out=e.cqb[:, ht, ci : ci + 1], in0=cq_ps[:, ht, ci : ci + 1],
                in1=e.cab[:, ht : ht + 1], op=OP.add)
    cw_ps = e.wps.tile([128, 2], F32, tag="work")
    for ci in range(2):
        for kc in range(3):
            nc.tensor.matmul(
                cw_ps[:, ci : ci + 1], e.s2w[:, kc, :],
                e.cvec[:, kc, ci : ci + 1],
                start=(ci == 0 and kc == 0), stop=(ci == 1 and kc == 2))
    nc.vector.memset(e.uband[:], 0.0)
    for ci in range(2):
        nc.vector.tensor_tensor(
            out=e.uband[:, 256 + ci : 257 + ci], in0=cw_ps[:, ci : ci + 1],
            in1=e.mlpw[:, 0:1], op=OP.mult)
    nc.scalar.copy(out=e.act_scr[0:1, 15:16], in_=e.cqb[0:1, 0, 0:1])
    e.tc.no_sync_barrier()


def _conv_stage(e, sg, xgs=None):
    """conv1+pool+conv2+pool for supergroup sg -> tmp2 [128, 2, 8, 32]."""
    nc = e.nc
    tmp2 = e.t2p.tile([128, 2, 8, L4], BF16, tag="tmp2")
    for g in range(2):
        if xgs is not None:
            xg = xgs[g]
        else:
            xg = e.xp.tile([128, 8, 4, 132], BF16, tag="xg")
            nc.sync.dma_start(out=xg[:], in_=e.io["x"][sg * 2 + g])

        t1 = e.t1p.tile([128, 4, 4, 68], BF16, tag="t1")
        nc.vector.memset(t1[:, :, :, 0:2], 0.0)
        nc.vector.memset(t1[:, :, :, 66:68], 0.0)
        for m in range(4):
            y1 = e.wps.tile([128, 4, 128], F32, tag="conv")
            n_mm = 0
            for kc in range(8):
                for t in range(5):
                    nc.tensor.matmul(
                        y1[:], e.c1wk[kc][:, t, m * 128 : (m + 1) * 128],
                        xg[:, kc, :, t : t + 128],
                        start=(n_mm == 0), stop=(n_mm == 39))
                    n_mm += 1
            ys = e.t1p.tile([128, 4, 128], F32, tag="pool1")
            nc.scalar.activation(out=ys[:], in_=y1[:], func=ACTF.Identity,
                                 bias=e.c1b[:, m : m + 1])
            yv = ys[:].rearrange("p s (u two) -> p s u two", two=2)
            nc.vector.tensor_tensor(
                out=t1[:, m, :, 2:66], in0=yv[:, :, :, 0],
                in1=yv[:, :, :, 1], op=OP.max)

        for m in range(2):
            y2 = e.wps.tile([128, 4, 64], F32, tag="conv")
            n_mm = 0
            for kc in range(4):
                for t in range(5):
                    nc.tensor.matmul(
                        y2[:], e.c2w[:, kc, t, m * 128 : (m + 1) * 128],
                        t1[:, kc, :, t : t + 64],
                        start=(n_mm == 0), stop=(n_mm == 19))
                    n_mm += 1
            ys = e.t1p.tile([128, 4, 64], F32, tag="pool2")
            nc.scalar.activation(out=ys[:], in_=y2[:], func=ACTF.Identity,
                                 bias=e.c2b[:, m : m + 1])
            yv = ys[:].rearrange("p s (u two) -> p s u two", two=2)
            nc.vector.tensor_tensor(
                out=tmp2[:, m, g * 4 : g * 4 + 4, :], in0=yv[:, :, :, 0],
                in1=yv[:, :, :, 1], op=OP.max)
    return tmp2


def _proj_stage(e, tmp2):
    """qp/kp projections (bf16 SBUF) + concept-key tanh tiles."""
    nc = e.nc
    qk_sb = e.atp.tile([128, 2, 2, 8, L4], BF16, tag="qk")  # [q/k, ht, ...]
    tcn = e.atp.tile([128, 2, 2, 8, L4], BF16, tag="tcn")   # [ht, ci, ...]
    for ht in range(2):
        qp_ps = e.wps.tile([128, 8, L4], F32, tag="conv")
        for kc in range(2):
            nc.tensor.matmul(
                qp_ps[:], e.saq[:, kc, ht * 128 : (ht + 1) * 128],
                tmp2[:, kc, :, :], start=(kc == 0), stop=(kc == 1))
        nc.vector.tensor_scalar_add(
            out=qk_sb[:, 0, ht, :, :], in0=qp_ps[:],
            scalar1=e.sbqk[:, ht : ht + 1])
        kp_ps = e.wps.tile([128, 8, L4], F32, tag="conv")
        for kc in range(2):
            nc.tensor.matmul(
                kp_ps[:], e.sak[:, kc, ht * 128 : (ht + 1) * 128],
                tmp2[:, kc, :, :], start=(kc == 0), stop=(kc == 1))
        nc.vector.tensor_copy(out=qk_sb[:, 1, ht, :, :], in_=kp_ps[:])
        ck = e.wps.tile([128, 8, L4], F32, tag="work")
        for kc in range(2):
            nc.tensor.matmul(
                ck[:], e.cak[:, kc, ht * 128 : (ht + 1) * 128],
                tmp2[:, kc, :, :], start=(kc == 0), stop=(kc == 1))
        for ci in range(2):
            nc.scalar.activation(
                out=tcn[:, ht, ci, :, :], in_=ck[:],
                func=ACTF.Tanh, bias=e.cqb[:, ht, ci : ci + 1])
    return qk_sb, tcn


def _attn_stage(e, sg, tmp2, qk_sb, tcn):
    nc = e.nc
    # tmp2 transposed: [k, seg, c]
    t2kc = e.t2p.tile([32, 8, C2], BF16, tag="t2kc")
    for sub in range(8):
        for m in range(2):
            tp = e.wps.tile([32, 128], BF16, tag="work")
            nc.tensor.transpose(tp[:], tmp2[:, m, sub, :], e.ident[:])
            nc.vector.tensor_copy(
                out=t2kc[:, sub, m * 128 : (m + 1) * 128], in_=tp[:])

    # additive scores, scattered to local psum rows 0..7 via banded v
    s_ps = e.sps.tile([8, 2, 512], F32, tag="s")      # [sub, half, qk]
    sc_ps = e.sps.tile([8, 2, L4], F32, tag="sc")     # [sub, ci, k]
    for sub in range(8):
        tts = e.bbp.tile([128, 2, 1024], BF16, tag="tts")
        for ht in range(2):
            bb = e.bbp.tile([128, L4, L4], BF16, tag="bb")
            nc.vector.tensor_tensor(
                out=bb[:],
                in0=qk_sb[:, 0, ht, sub, :].unsqueeze(2).broadcast_to(
                    [128, L4, L4]),
                in1=qk_sb[:, 1, ht, sub, :].unsqueeze(1).broadcast_to(
                    [128, L4, L4]),
                op=OP.add)
            nc.scalar.activation(
                out=tts[:, ht, :].rearrange("p (q k) -> p q k", k=L4),
                in_=bb[:], func=ACTF.Tanh)
        for half in range(2):
            for ht in range(2):
                nc.tensor.matmul(
                    s_ps[:, half, :],
                    e.vband[:, ht, 128 - sub : 136 - sub],
                    tts[:, ht, half * 512 : (half + 1) * 512],
                    start=(ht == 0 and sub == 0),
                    stop=(ht == 1 and sub == 7))
        for ht in range(2):
            nc.tensor.matmul(
                sc_ps[:, :, :],
                e.cavband[:, ht, 128 - sub : 136 - sub],
                tcn[:, ht, :, sub, :],
                start=(ht == 0 and sub == 0),
                stop=(ht == 1 and sub == 7))

    # masked softmaxes (rows 0..7)
    sv = s_ps[:].rearrange("s h (q k) -> s (h q) k", k=L4)
    nc.vector.tensor_tensor(
        out=sv[:], in0=sv[:],
        in1=e.mb[:, sg, :].unsqueeze(1).broadcast_to([8, L4, L4]),
        op=OP.add)
    nc.scalar.activation(out=sv[:], in_=sv[:], func=ACTF.Exp)
    zs = e.smp.tile([8, L4], F32, tag="zs")
    nc.vector.reduce_sum(out=zs[:], in_=sv[:], axis=AX.X)
    nc.vector.reciprocal(out=zs[:], in_=zs[:])
    a_sb = e.smp.tile([8, L4, L4], BF16, tag="a_sb")
    nc.vector.tensor_tensor(
        out=a_sb[:], in0=sv[:],
        in1=zs[:].unsqueeze(2).broadcast_to([8, L4, L4]), op=OP.mult)

    for ci in range(2):
        nc.vector.tensor_tensor(
            out=sc_ps[:, ci, :], in0=sc_ps[:, ci, :],
            in1=e.mb[:, sg, :], op=OP.add)
    nc.scalar.activation(out=sc_ps[:], in_=sc_ps[:], func=ACTF.Exp)
    zc = e.smp.tile([8, 2], F32, tag="zc")
    nc.vector.reduce_sum(out=zc[:], in_=sc_ps[:], axis=AX.X)
    nc.vector.reciprocal(out=zc[:], in_=zc[:])
    ac_sb = e.smp.tile([8, 2, L4], BF16, tag="ac_sb")
    nc.vector.tensor_tensor(
        out=ac_sb[:], in0=sc_ps[:],
        in1=zc[:].unsqueeze(2).broadcast_to([8, 2, L4]), op=OP.mult)

    # transpose attention weights: k onto partitions
    aT_ps = e.wps.tile([32, L4, 8], BF16, tag="work")
    for q in range(L4):
        nc.tensor.matmul(
            aT_ps[:, q, :], a_sb[:, q, :], e.ident[0:8, 0:8],
            is_transpose=True, start=(q == 0), stop=(q == L4 - 1))
    aT = e.atp.tile([32, L4, 8], BF16, tag="aTs")     # [k, q, sub]
    nc.vector.tensor_copy(out=aT[:], in_=aT_ps[:])
    acT_ps = e.wps.tile([32, 2, 8], BF16, tag="work")
    for ci in range(2):
        nc.tensor.matmul(
            acT_ps[:, ci, :], ac_sb[:, ci, :], e.ident[0:8, 0:8],
            is_transpose=True, start=(ci == 0), stop=(ci == 1))
    acT = e.atp.tile([32, 2, 8], BF16, tag="acTs")    # [k, ci, sub]
    nc.vector.tensor_copy(out=acT[:], in_=acT_ps[:])

    # per-segment: self_res, concept_res, concat
    ar = e.arp.tile([128, 8, 8, 34], BF16, tag="ar")
    nc.vector.memset(ar[:, :, :, 0:1], 0.0)
    nc.vector.memset(ar[:, :, :, 33:34], 0.0)
    for m in range(2):
        nc.vector.tensor_copy(out=ar[:, m, :, 1:33], in_=tmp2[:, m, :, :])
    for sub in range(8):
        srp = e.wps.tile([128, 2, 34], F32, tag="work")
        n_mm = 0
        for m in range(2):
            for qh in range(2):
                nc.tensor.matmul(
                    srp[:, m, qh * 16 : (qh + 1) * 16],
                    t2kc[:, sub, m * 128 : (m + 1) * 128],
                    aT[:, qh * 16 : (qh + 1) * 16, sub],
                    start=(n_mm == 0), stop=(n_mm == 7))
                n_mm += 1
            for ci in range(2):
                nc.tensor.matmul(
                    srp[:, m, 32 + ci : 33 + ci],
                    t2kc[:, sub, m * 128 : (m + 1) * 128],
                    acT[:, ci, sub : sub + 1],
                    start=(n_mm == 0), stop=(n_mm == 7))
                n_mm += 1
        nc.vector.tensor_copy(out=ar[:, 2:4, sub, 1:33], in_=srp[:, :, 0:32])
        for ci in range(2):
            for m in range(2):
                nc.vector.tensor_copy(
                    out=ar[:, 4 + 2 * ci + m, sub, 1:33],
                    in_=srp[:, m, 32 + ci : 33 + ci].broadcast_to([128, L4]))
    return ar


def _deconv_score_stage(e, sg, ar):
    nc = e.nc
    # deconv1: [1024,32] -> [512,64]
    r1t = e.r1p.tile([128, 4, 8, 66], BF16, tag="r1t")
    nc.vector.memset(r1t[:, :, :, 0:1], 0.0)
    nc.vector.memset(r1t[:, :, :, 65:66], 0.0)
    for m in range(4):
        for par, taps in DC_TAPS:
            d1 = e.wps.tile([128, 8, L4], F32, tag="work")
            n_mm = 0
            for kc in range(8):
                for t, off in taps:
                    nc.tensor.matmul(
                        d1[:], e.dc1w[:, kc, t, m * 128 : (m + 1) * 128],
                        ar[:, kc, :, off : off + 32],
                        start=(n_mm == 0), stop=(n_mm == 15))
                    n_mm += 1
            nc.vector.tensor_scalar_add(
                out=r1t[:, m, :, 1 + par : 65 + par : 2], in0=d1[:],
                scalar1=e.dc1b[:, m : m + 1])

    # deconv2: [512,64] -> [128,128]
    r2t = e.r2p.tile([128, 8, 128], BF16, tag="r2t")
    for par, taps in DC_TAPS:
        d2 = e.wps.tile([128, 8, 64], F32, tag="work")
        n_mm = 0
        for kc in range(4):
            for t, off in taps:
                nc.tensor.matmul(
                    d2[:], e.dc2w[:, kc, t, :],
                    r1t[:, kc, :, off : off + 64],
                    start=(n_mm == 0), stop=(n_mm == 7))
                n_mm += 1
        nc.vector.tensor_scalar_add(
            out=r2t[:, :, par : 128 : 2], in0=d2[:], scalar1=e.dc2b[:, 0:1])

    # scoring
    score_ps = e.scp.tile([16, 128], F32, tag="score")   # [(sub,ci), l]
    for sub in range(8):
        sim_ps = e.wps.tile([128, 128], F32, tag="work")
        nc.tensor.matmul(sim_ps[:], e.s1w[:], r2t[:, sub, :],
                         start=True, stop=True)
        sim_sb = e.smp.tile([128, 128], BF16, tag="sim_sb")
        nc.vector.tensor_copy(out=sim_sb[:], in_=sim_ps[:])
        nc.tensor.matmul(
            score_ps[:],
            e.uband[:, (128 - sub) * 2 : (128 - sub) * 2 + 16], sim_sb[:],
            start=(sub == 0), stop=(sub == 7))
    final = e.finp.tile([16, 128], F32, tag="final")
    nc.scalar.activation(out=final[:], in_=score_ps[:], func=ACTF.Sigmoid,
                         bias=e.mlpb[:, 0:1])
    nc.sync.dma_start(
        out=e.io["out"].transpose([1, 0, 2])[sg * 8 : sg * 8 + 8],
        in_=final[:])


def _late_weights(e):
    nc = e.nc
    for t_sb, name in [(e.dc1w, "dc1w"), (e.dc2w, "dc2w"), (e.s1w, "s1w")]:
        nc.sync.dma_start(out=t_sb[:], in_=e.io[name])


def _emit(ctx, tc, io, reps=1):
    e = _setup(ctx, tc, io)
    for _rep in range(reps):
        _phase0(e)
        tmp2_cur = _conv_stage(e, 0, xgs=e.xg0 if _rep == 0 else None)
        if _rep == 0:
            _late_weights(e)
        for sg in range(4):
            qk_sb, tcn = _proj_stage(e, tmp2_cur)
            tmp2_next = _conv_stage(e, sg + 1) if sg < 3 else None
            ar = _attn_stage(e, sg, tmp2_cur, qk_sb, tcn)
            _deconv_score_stage(e, sg, ar)
            tmp2_cur = tmp2_next


# ---------------------------------------------------------------------------
# program build (cached)
# ---------------------------------------------------------------------------

_CACHE = {}


def _build(reps=1):
    key = ("nc", reps)
    if key in _CACHE:
        return _CACHE[key]
    nc = bacc.Bacc("TRN2", target_bir_lowering=False, debug=False)
    d = {}

    def di(name, shape, dt):
        d[name] = nc.dram_tensor(name, shape, dt, kind="ExternalInput").ap()

    di("x", [8, 128, 8, 4, 132], BF16)
    di("c1w", [128, 8, 5, C1], BF16)
    di("c2w", [128, 4, 5, C2], BF16)
    di("dc1w", [128, 8, 4, D1], BF16)
    di("dc2w", [128, 4, 4, D2], BF16)
    di("saq", [128, 2, C2], BF16)
    di("sak", [128, 2, C2], BF16)
    di("cak", [128, 2, C2], BF16)
    di("caq", [128, 3, C2], BF16)
    di("s1w", [128, SIM], BF16)
    di("s2w", [128, 3, SIM], BF16)
    di("vband", [128, 2, 256], BF16)
    di("cavband", [128, 2, 256], BF16)
    di("c1b", [128, 4], F32)
    di("c2b", [128, 2], F32)
    di("dc1b", [128, 4], F32)
    di("dc2b", [128, 1], F32)
    di("sbqk", [128, 2], F32)
    di("cab", [128, 2], F32)
    di("mlpw", [128, 1], F32)
    di("mlpb", [16, 1], F32)
    di("cvec", [128, 3, 2], BF16)
    di("mb", [8, 4, L4], F32)
    d["out"] = nc.dram_tensor("out", [2, SEG, 128], F32,
                              kind="ExternalOutput").ap()
    with tile.TileContext(nc) as tc:
        with ExitStack() as ctx:
            _emit(ctx, tc, d, reps=reps)
    nc.compile()
    _CACHE[key] = nc
    return nc


# ---------------------------------------------------------------------------
# host-side prep (layout/cast only)
# ---------------------------------------------------------------------------

def _chunk_bias(v, nchunk):
    return np.ascontiguousarray(
        np.asarray(v, np.float32).reshape(nchunk, 128).T)


def _band(v):
    # [128, 2, 256] bf16; column 128 of chunk ht = v[ht*128:(ht+1)*128]
    out = np.zeros((128, 2, 256), nbf)
    vv = np.asarray(v, np.float32).reshape(2, 128).T
    out[:, :, 128] = vv.astype(nbf)
    return out


def _wchunks(w, nk):
    # w: [K, ...] -> [128, nk, ...] (zero-pad K up to nk*128)
    w = np.asarray(w, np.float32)
    k = w.shape[0]
    if k < nk * 128:
        w = np.concatenate(
            [w, np.zeros((nk * 128 - k,) + w.shape[1:], np.float32)], 0)
    w = w.reshape((nk, 128) + w.shape[1:])
    perm = (1, 0) + tuple(range(2, w.ndim))
    return np.ascontiguousarray(w.transpose(perm)).astype(nbf)


def prepare_common(inp):
    g = {}
    g["c1w"] = _wchunks(np.asarray(inp["conv1_w"], np.float32)
                        .transpose(1, 2, 0), 8)       # [128,8,5,512]
    g["c2w"] = _wchunks(np.asarray(inp["conv2_w"], np.float32)
                        .transpose(1, 2, 0), 4)       # [128,4,5,256]
    g["dc1w"] = _wchunks(np.asarray(inp["dc1_w"], np.float32)
                         .transpose(0, 2, 1), 8)      # [128,8,4,512]
    g["dc2w"] = _wchunks(np.asarray(inp["dc2_w"], np.float32)
                         .transpose(0, 2, 1), 4)      # [128,4,4,128]
    g["saq"] = _wchunks(inp["sa_wq"], 2)
    g["sak"] = _wchunks(inp["sa_wk"], 2)
    g["cak"] = _wchunks(inp["ca_wk"], 2)
    g["caq"] = _wchunks(inp["ca_wq"], 3)              # [128,3,256]
    g["s1w"] = np.ascontiguousarray(
        np.asarray(inp["sim1_w"], np.float32)).astype(nbf)
    g["s2w"] = _wchunks(inp["sim2_w"], 3)             # [128,3,128]
    g["vband"] = _band(inp["sa_v"])
    g["cavband"] = _band(inp["ca_v"])
    g["c1b"] = _chunk_bias(inp["conv1_b"], 4)
    g["c2b"] = _chunk_bias(inp["conv2_b"], 2)
    g["dc1b"] = _chunk_bias(inp["dc1_b"], 4)
    g["dc2b"] = _chunk_bias(inp["dc2_b"], 1)
    g["sbqk"] = _chunk_bias(
        np.asarray(inp["sa_bq"], np.float32)
        + np.asarray(inp["sa_bk"], np.float32), 2)
    g["cab"] = _chunk_bias(
        np.asarray(inp["ca_bq"], np.float32)
        + np.asarray(inp["ca_bk"], np.float32), 2)
    g["mlpw"] = np.ascontiguousarray(
        np.asarray(inp["mlp_w"], np.float32).reshape(128, 1))
    g["mlpb"] = np.full((16, 1), float(np.asarray(inp["mlp_b"])), np.float32)
    return g


def prepare_core(inp, b):
    o = {}
    x = np.asarray(inp["batch"], np.float32)[b]       # [32,128,1024]
    x = x.transpose(0, 2, 1)                          # [32,1024,128]
    xp = np.zeros((SEG, CIN, 132), np.float32)
    xp[:, :, 2:130] = x
    xp = xp.reshape(8, 4, 8, 128, 132).transpose(0, 3, 2, 1, 4)
    o["x"] = np.ascontiguousarray(xp).astype(nbf)     # [8,128,8,4,132]
    cv = np.zeros((2, 384), np.float32)
    cv[0, :CD] = np.asarray(inp["concept1"], np.float32)[b]
    cv[1, :CD] = np.asarray(inp["concept2"], np.float32)[b]
    o["cvec"] = np.ascontiguousarray(
        cv.reshape(2, 3, 128).transpose(2, 1, 0)).astype(nbf)  # [128,3,2]
    sl = np.asarray(inp["seg_len"], np.int64)[b]      # [32]
    k = np.arange(L4)
    m = np.where(sl[:, None] > 4 * k[None, :], 0.0, NEG).astype(np.float32)
    o["mb"] = np.ascontiguousarray(
        m.reshape(4, 8, L4).transpose(1, 0, 2))       # [8, 4, 32]
    return o


def kernel(**inputs):
    nc = _build()
    common = prepare_common(inputs)
    in_maps = []
    for b in range(B):
        m = dict(common)
        m.update(prepare_core(inputs, b))
        in_maps.append(m)
    res = run_bass_kernel_spmd(nc, in_maps, list(range(B)))
    s1 = np.stack([res.results[b]["out"][0] for b in range(B)])
    s2 = np.stack([res.results[b]["out"][1] for b in range(B)])
    return s1.astype(np.float32), s2.astype(np.float32)


# revision 53
# speedup vs baseline: 159.4804x; 1.0732x over previous
"""Trainium2 Bass kernel for nn_CHAN_29764123361280 (ragged_sequence).

Sharding: data-parallel over the batch axis B=8 -> one video per NeuronCore,
all weights replicated. Per-core pipeline (32 segments):
  conv1(k5,p2)+maxpool2 -> conv2(k5,p2)+maxpool2 -> additive self-attention
  + two concept attentions -> concat -> deconv1 -> deconv2 -> similarity
  scoring.  All matmuls bf16 with fp32 PSUM accumulation.

Per-segment additive-attention scores are packed into PSUM rows with a
"banded v" trick: lhsT is a [128,8] slice of a banded matrix whose only
nonzero column is sa_v placed so that segment-local row `sub` receives
v . tanh(qp+kp), accumulated over both 128-chunks of the hidden dim.

The conv stage for supergroup sg+1 is emitted between the projection and
attention phases of supergroup sg so the PE has conv work while the
DVE/ACT tanh chain runs.
"""

from contextlib import ExitStack

import numpy as np
import ml_dtypes

import concourse.bass as bass  # noqa: F401
import concourse.mybir as mybir
import concourse.tile as tile
from concourse import bacc
from concourse.bass_utils import run_bass_kernel_spmd
from concourse.masks import make_identity

BF16 = mybir.dt.bfloat16
F32 = mybir.dt.float32

B, S, L, CIN = 8, 32, 128, 1024
C1, C2 = 512, 256
D1, D2 = 512, 128
CD, SIM = 300, 128
L4 = L // 4          # 32
SEG = S              # segments per core
NEG = -30.0          # mask logit bias (exp(-30) ~ 1e-13)

nbf = ml_dtypes.bfloat16

DC_TAPS = ((0, ((1, 1), (3, 0))), (1, ((2, 1), (0, 2))))
# parity -> ((tap, input col offset), ...) for ConvTranspose1d(k=4,s=2,p=1)
# on halo'd input xh[:, u+1] = x[:, u]:
#   even out j=2u:  W1.x[u]  + W3.x[u-1]
#   odd  out j=2u+1: W2.x[u] + W0.x[u+1]

AX = mybir.AxisListType
OP = mybir.AluOpType
ACTF = mybir.ActivationFunctionType


class _Env:
    pass


def _setup(ctx, tc, io):
    nc = tc.nc
    e = _Env()
    e.nc, e.tc, e.io = nc, tc, io
    singles = ctx.enter_context(tc.tile_pool(name="singles", bufs=1))
    e.singles = singles
    # ---- resident weights / constants ----
    e.c1wk = [singles.tile([128, 5, C1], BF16, name=f"c1w{i}",
                           tag=f"c1w{i}") for i in range(8)]
    e.c2w = singles.tile([128, 4, 5, C2], BF16)
    e.dc1w = singles.tile([128, 8, 4, D1], BF16)
    e.dc2w = singles.tile([128, 4, 4, D2], BF16)
    e.saq = singles.tile([128, 2, C2], BF16)
    e.sak = singles.tile([128, 2, C2], BF16)
    e.cak = singles.tile([128, 2, C2], BF16)
    e.caq = singles.tile([128, 3, C2], BF16)
    e.s1w = singles.tile([128, SIM], BF16)
    e.s2w = singles.tile([128, 3, SIM], BF16)
    e.vband = singles.tile([128, 2, 256], BF16)
    e.cavband = singles.tile([128, 2, 256], BF16)
    e.c1b = singles.tile([128, 4], F32)
    e.c2b = singles.tile([128, 2], F32)
    e.dc1b = singles.tile([128, 4], F32)
    e.dc2b = singles.tile([128, 1], F32)
    e.sbqk = singles.tile([128, 2], F32)
    e.cab = singles.tile([128, 2], F32)
    e.mlpw = singles.tile([128, 1], F32)
    e.mlpb = singles.tile([16, 1], F32)
    e.cvec = singles.tile([128, 3, 2], BF16)
    e.mb = singles.tile([8, 4, L4], F32)    # [local row, supergroup, k]
    e.ident = singles.tile([128, 128], BF16)

    # ---- pools ----
    e.xp = ctx.enter_context(tc.tile_pool(name="xp", bufs=3))
    e.t1p = ctx.enter_context(tc.tile_pool(name="t1p", bufs=3))
    e.t2p = ctx.enter_context(tc.tile_pool(name="t2p", bufs=2))
    e.atp = ctx.enter_context(tc.tile_pool(name="atp", bufs=2))
    e.bbp = ctx.enter_context(tc.tile_pool(name="bbp", bufs=3))
    e.arp = ctx.enter_context(tc.tile_pool(name="arp", bufs=2))
    e.r1p = ctx.enter_context(tc.tile_pool(name="r1p", bufs=2))
    e.r2p = ctx.enter_context(tc.tile_pool(name="r2p", bufs=2))
    e.smp = ctx.enter_context(tc.tile_pool(name="smp", bufs=3))
    e.php = ctx.enter_context(tc.tile_pool(name="php", bufs=1))
    e.finp = ctx.enter_context(tc.tile_pool(name="finp", bufs=2))

    # PSUM: conv(2x1) + work(2x1) + s(2) + sc(1) + score(1) = 8 banks
    e.wps = ctx.enter_context(tc.tile_pool(name="wps", bufs=2, space="PSUM"))
    e.sps = ctx.enter_context(tc.tile_pool(name="sps", bufs=1, space="PSUM"))
    e.scp = ctx.enter_context(tc.tile_pool(name="scp", bufs=1, space="PSUM"))

    e.xg0 = []
    for g in range(2):
        xg = e.xp.tile([128, 8, 4, 132], BF16, name=f"xg0{g}", tag="xg")
        nc.sync.dma_start(out=xg[:], in_=io["x"][g])
        e.xg0.append(xg)
    for t_sb, name in [
        (e.cvec, "cvec"), (e.caq, "caq"), (e.s2w, "s2w"),
        (e.c1b, "c1b"), (e.c2b, "c2b"), (e.dc1b, "dc1b"), (e.dc2b, "dc2b"),
        (e.sbqk, "sbqk"), (e.cab, "cab"), (e.mlpw, "mlpw"),
        (e.mlpb, "mlpb"), (e.mb, "mb"),
        (e.vband, "vband"), (e.cavband, "cavband"),
    ]:
        nc.sync.dma_start(out=t_sb[:], in_=io[name])
    for i in range(8):
        nc.sync.dma_start(out=e.c1wk[i][:], in_=io["c1w"][:, i])
    nc.sync.dma_start(out=e.c2w[:], in_=io["c2w"])
    nc.sync.dma_start(out=e.saq[:], in_=io["saq"])
    nc.sync.dma_start(out=e.sak[:], in_=io["sak"])
    nc.sync.dma_start(out=e.cak[:], in_=io["cak"])
    make_identity(nc, e.ident[:])

    # Touch every DMA'd tensor an engine will read, one instruction per
    # tensor, so each engine's vector clock observes the DMA semaphores
    # early: later compute then never needs >1 sync wait per instruction
    # (the walrus TT/STT encodings only carry one).
    e.dve_scr = singles.tile([1, 16], F32)
    e.act_scr = singles.tile([1, 16], F32)
    for i, t_sb in enumerate(
            (e.cab, e.mlpw, e.mb[:, 0, :], e.dc1b, e.dc2b, e.sbqk,
             e.c1b, e.c2b, e.mlpb)):
        nc.vector.tensor_copy(out=e.dve_scr[0:1, i : i + 1],
                              in_=t_sb[0:1, 0:1])
        nc.scalar.copy(out=e.act_scr[0:1, i : i + 1], in_=t_sb[0:1, 0:1])

    return e


def _phase0(e):
    nc = e.nc
    e.cqb = e.php.tile([128, 2, 2], F32)       # tanh bias for concept attn
    e.uband = e.php.tile([128, 512], BF16)     # banded (ci-interleaved) u
    cq_ps = e.wps.tile([128, 2, 2], F32, tag="work")
    for i, (ci, ht) in enumerate([(c, h) for c in range(2) for h in range(2)]):
        for kc in range(3):
            nc.tensor.matmul(
                cq_ps[:, ht, ci : ci + 1],
                e.caq[:, kc, ht * 128 : (ht + 1) * 128],
                e.cvec[:, kc, ci : ci + 1],
                start=(i == 0 and kc == 0), stop=(i == 3 and kc == 2))
    for ci in range(2):
        for ht in range(2):
            nc.vector.tensor_tensor(
                out=e.cqb[:, ht, ci : ci + 1], in0=cq_ps[:, ht, ci : ci + 1],
                in1=e.cab[:, ht : ht + 1], op=OP.add)
    cw_ps = e.wps.tile([128, 2], F32, tag="work")
    for ci in range(2):
        for kc in range(3):
            nc.tensor.matmul(
                cw_ps[:, ci : ci + 1], e.s2w[:, kc, :],
                e.cvec[:, kc, ci : ci + 1],
                start=(ci == 0 and kc == 0), stop=(ci == 1 and kc == 2))
    nc.vector.memset(e.uband[:], 0.0)
    for ci in range(2):
        nc.vector.tensor_tensor(
            out=e.uband[:, 256 + ci : 257 + ci], in0=cw_ps[:, ci : ci + 1],
            in1=e.mlpw[:, 0:1], op=OP.mult)
    nc.scalar.copy(out=e.act_scr[0:1, 15:16], in_=e.cqb[0:1, 0, 0:1])
    e.tc.no_sync_barrier()


def _conv_stage(e, sg, xgs=None):
    """conv1+pool+conv2+pool for supergroup sg -> tmp2 [128, 2, 8, 32]."""
    nc = e.nc
    tmp2 = e.t2p.tile([128, 2, 8, L4], BF16, tag="tmp2")
    for g in range(2):
        if xgs is not None:
            xg = xgs[g]
        else:
            xg = e.xp.tile([128, 8, 4, 132], BF16, tag="xg")
            nc.sync.dma_start(out=xg[:], in_=e.io["x"][sg * 2 + g])

        t1 = e.t1p.tile([128, 4, 4, 68], BF16, tag="t1")
        nc.vector.memset(t1[:, :, :, 0:2], 0.0)
        nc.vector.memset(t1[:, :, :, 66:68], 0.0)
        for m in range(4):
            y1 = e.wps.tile([128, 4, 128], F32, tag="conv")
            n_mm = 0
            for kc in range(8):
                for t in range(5):
                    nc.tensor.matmul(
                        y1[:], e.c1wk[kc][:, t, m * 128 : (m + 1) * 128],
                        xg[:, kc, :, t : t + 128],
                        start=(n_mm == 0), stop=(n_mm == 39))
                    n_mm += 1
            ys = e.t1p.tile([128, 4, 128], F32, tag="pool1")
            nc.scalar.activation(out=ys[:], in_=y1[:], func=ACTF.Identity,
                                 bias=e.c1b[:, m : m + 1])
            yv = ys[:].rearrange("p s (u two) -> p s u two", two=2)
            nc.vector.tensor_tensor(
                out=t1[:, m, :, 2:66], in0=yv[:, :, :, 0],
                in1=yv[:, :, :, 1], op=OP.max)

        for m in range(2):
            y2 = e.wps.tile([128, 4, 64], F32, tag="conv")
            n_mm = 0
            for kc in range(4):
                for t in range(5):
                    nc.tensor.matmul(
                        y2[:], e.c2w[:, kc, t, m * 128 : (m + 1) * 128],
                        t1[:, kc, :, t : t + 64],
                        start=(n_mm == 0), stop=(n_mm == 19))
                    n_mm += 1
            ys = e.t1p.tile([128, 4, 64], F32, tag="pool2")
            nc.scalar.activation(out=ys[:], in_=y2[:], func=ACTF.Identity,
                                 bias=e.c2b[:, m : m + 1])
            yv = ys[:].rearrange("p s (u two) -> p s u two", two=2)
            nc.vector.tensor_tensor(
                out=tmp2[:, m, g * 4 : g * 4 + 4, :], in0=yv[:, :, :, 0],
                in1=yv[:, :, :, 1], op=OP.max)
    return tmp2


def _proj_stage(e, tmp2):
    """qp/kp projections (bf16 SBUF) + concept-key tanh tiles."""
    nc = e.nc
    qk_sb = e.atp.tile([128, 2, 2, 8, L4], BF16, tag="qk")  # [q/k, ht, ...]
    tcn = e.atp.tile([128, 2, 2, 8, L4], BF16, tag="tcn")   # [ht, ci, ...]
    for ht in range(2):
        qp_ps = e.wps.tile([128, 8, L4], F32, tag="conv")
        for kc in range(2):
            nc.tensor.matmul(
                qp_ps[:], e.saq[:, kc, ht * 128 : (ht + 1) * 128],
                tmp2[:, kc, :, :], start=(kc == 0), stop=(kc == 1))
        nc.vector.tensor_scalar_add(
            out=qk_sb[:, 0, ht, :, :], in0=qp_ps[:],
            scalar1=e.sbqk[:, ht : ht + 1])
        kp_ps = e.wps.tile([128, 8, L4], F32, tag="conv")
        for kc in range(2):
            nc.tensor.matmul(
                kp_ps[:], e.sak[:, kc, ht * 128 : (ht + 1) * 128],
                tmp2[:, kc, :, :], start=(kc == 0), stop=(kc == 1))
        nc.vector.tensor_copy(out=qk_sb[:, 1, ht, :, :], in_=kp_ps[:])
        ck = e.wps.tile([128, 8, L4], F32, tag="work")
        for kc in range(2):
            nc.tensor.matmul(
                ck[:], e.cak[:, kc, ht * 128 : (ht + 1) * 128],
                tmp2[:, kc, :, :], start=(kc == 0), stop=(kc == 1))
        for ci in range(2):
            nc.scalar.activation(
                out=tcn[:, ht, ci, :, :], in_=ck[:],
                func=ACTF.Tanh, bias=e.cqb[:, ht, ci : ci + 1])
    return qk_sb, tcn


def _attn_stage(e, sg, tmp2, qk_sb, tcn):
    nc = e.nc
    # tmp2 transposed: [k, seg, c]
    t2kc = e.t2p.tile([32, 8, C2], BF16, tag="t2kc")
    for sub in range(8):
        for m in range(2):
            tp = e.wps.tile([32, 128], BF16, tag="work")
            nc.tensor.transpose(tp[:], tmp2[:, m, sub, :], e.ident[:])
            nc.vector.tensor_copy(
                out=t2kc[:, sub, m * 128 : (m + 1) * 128], in_=tp[:])

    # additive scores, scattered to local psum rows 0..7 via banded v
    s_ps = e.sps.tile([8, 2, 512], F32, tag="s")      # [sub, half, qk]
    sc_ps = e.sps.tile([8, 2, L4], F32, tag="sc")     # [sub, ci, k]
    for sub in range(8):
        tts = e.bbp.tile([128, 2, 1024], BF16, tag="tts")
        for ht in range(2):
            bb = e.bbp.tile([128, L4, L4], BF16, tag="bb")
            nc.vector.tensor_tensor(
                out=bb[:],
                in0=qk_sb[:, 0, ht, sub, :].unsqueeze(2).broadcast_to(
                    [128, L4, L4]),
                in1=qk_sb[:, 1, ht, sub, :].unsqueeze(1).broadcast_to(
                    [128, L4, L4]),
                op=OP.add)
            nc.scalar.activation(
                out=tts[:, ht, :].rearrange("p (q k) -> p q k", k=L4),
                in_=bb[:], func=ACTF.Tanh)
        for half in range(2):
            for ht in range(2):
                nc.tensor.matmul(
                    s_ps[:, half, :],
                    e.vband[:, ht, 128 - sub : 136 - sub],
                    tts[:, ht, half * 512 : (half + 1) * 512],
                    start=(ht == 0 and sub == 0),
                    stop=(ht == 1 and sub == 7))
        for ht in range(2):
            nc.tensor.matmul(
                sc_ps[:, :, :],
                e.cavband[:, ht, 128 - sub : 136 - sub],
                tcn[:, ht, :, sub, :],
                start=(ht == 0 and sub == 0),
                stop=(ht == 1 and sub == 7))

    # masked softmaxes (rows 0..7)
    sv = s_ps[:].rearrange("s h (q k) -> s (h q) k", k=L4)
    nc.vector.tensor_tensor(
        out=sv[:], in0=sv[:],
        in1=e.mb[:, sg, :].unsqueeze(1).broadcast_to([8, L4, L4]),
        op=OP.add)
    nc.scalar.activation(out=sv[:], in_=sv[:], func=ACTF.Exp)
    zs = e.smp.tile([8, L4], F32, tag="zs")
    nc.vector.reduce_sum(out=zs[:], in_=sv[:], axis=AX.X)
    nc.vector.reciprocal(out=zs[:], in_=zs[:])
    a_sb = e.smp.tile([8, L4, L4], BF16, tag="a_sb")
    nc.vector.tensor_tensor(
        out=a_sb[:], in0=sv[:],
        in1=zs[:].unsqueeze(2).broadcast_to([8, L4, L4]), op=OP.mult)

    for ci in range(2):
        nc.vector.tensor_tensor(
            out=sc_ps[:, ci, :], in0=sc_ps[:, ci, :],
            in1=e.mb[:, sg, :], op=OP.add)
    nc.scalar.activation(out=sc_ps[:], in_=sc_ps[:], func=ACTF.Exp)
    zc = e.smp.tile([8, 2], F32, tag="zc")
    nc.vector.reduce_sum(out=zc[:], in_=sc_ps[:], axis=AX.X)
    nc.vector.reciprocal(out=zc[:], in_=zc[:])
    ac_sb = e.smp.tile([8, 2, L4], BF16, tag="ac_sb")
    nc.vector.tensor_tensor(
        out=ac_sb[:], in0=sc_ps[:],
        in1=zc[:].unsqueeze(2).broadcast_to([8, 2, L4]), op=OP.mult)

    # transpose attention weights: k onto partitions
    aT_ps = e.wps.tile([32, L4, 8], BF16, tag="work")
    for q in range(L4):
        nc.tensor.matmul(
            aT_ps[:, q, :], a_sb[:, q, :], e.ident[0:8, 0:8],
            is_transpose=True, start=(q == 0), stop=(q == L4 - 1))
    aT = e.atp.tile([32, L4, 8], BF16, tag="aTs")     # [k, q, sub]
    nc.vector.tensor_copy(out=aT[:], in_=aT_ps[:])
    acT_ps = e.wps.tile([32, 2, 8], BF16, tag="work")
    for ci in range(2):
        nc.tensor.matmul(
            acT_ps[:, ci, :], ac_sb[:, ci, :], e.ident[0:8, 0:8],
            is_transpose=True, start=(ci == 0), stop=(ci == 1))
    acT = e.atp.tile([32, 2, 8], BF16, tag="acTs")    # [k, ci, sub]
    nc.vector.tensor_copy(out=acT[:], in_=acT_ps[:])

    # per-segment: self_res, concept_res, concat
    ar = e.arp.tile([128, 8, 8, 34], BF16, tag="ar")
    nc.vector.memset(ar[:, :, :, 0:1], 0.0)
    nc.vector.memset(ar[:, :, :, 33:34], 0.0)
    for m in range(2):
        nc.vector.tensor_copy(out=ar[:, m, :, 1:33], in_=tmp2[:, m, :, :])
    for sub in range(8):
        srp = e.wps.tile([128, 2, 34], F32, tag="work")
        n_mm = 0
        for m in range(2):
            for qh in range(2):
                nc.tensor.matmul(
                    srp[:, m, qh * 16 : (qh + 1) * 16],
                    t2kc[:, sub, m * 128 : (m + 1) * 128],
                    aT[:, qh * 16 : (qh + 1) * 16, sub],
                    start=(n_mm == 0), stop=(n_mm == 7))
                n_mm += 1
            for ci in range(2):
                nc.tensor.matmul(
                    srp[:, m, 32 + ci : 33 + ci],
                    t2kc[:, sub, m * 128 : (m + 1) * 128],
                    acT[:, ci, sub : sub + 1],
                    start=(n_mm == 0), stop=(n_mm == 7))
                n_mm += 1
        nc.vector.tensor_copy(out=ar[:, 2:4, sub, 1:33], in_=srp[:, :, 0:32])
        for ci in range(2):
            for m in range(2):
                nc.vector.tensor_copy(
                    out=ar[:, 4 + 2 * ci + m, sub, 1:33],
                    in_=srp[:, m, 32 + ci : 33 + ci].broadcast_to([128, L4]))
    return ar


def _deconv_score_stage(e, sg, ar):
    nc = e.nc
    # deconv1: [1024,32] -> [512,64]
    r1t = e.r1p.tile([128, 4, 8, 66], BF16, tag="r1t")
    nc.vector.memset(r1t[:, :, :, 0:1], 0.0)
    nc.vector.memset(r1t[:, :, :, 65:66], 0.0)
    for m in range(4):
        for par, taps in DC_TAPS:
            d1 = e.wps.tile([128, 8, L4], F32, tag="work")
            n_mm = 0
            for kc in range(8):
                for t, off in taps:
                    nc.tensor.matmul(
                        d1[:], e.dc1w[:, kc, t, m * 128 : (m + 1) * 128],
                        ar[:, kc, :, off : off + 32],
                        start=(n_mm == 0), stop=(n_mm == 15))
                    n_mm += 1
            nc.vector.tensor_scalar_add(
                out=r1t[:, m, :, 1 + par : 65 + par : 2], in0=d1[:],
                scalar1=e.dc1b[:, m : m + 1])

    # deconv2: [512,64] -> [128,128]
    r2t = e.r2p.tile([128, 8, 128], BF16, tag="r2t")
    for par, taps in DC_TAPS:
        d2 = e.wps.tile([128, 8, 64], F32, tag="work")
        n_mm = 0
        for kc in range(4):
            for t, off in taps:
                nc.tensor.matmul(
                    d2[:], e.dc2w[:, kc, t, :],
                    r1t[:, kc, :, off : off + 64],
                    start=(n_mm == 0), stop=(n_mm == 7))
                n_mm += 1
        nc.vector.tensor_scalar_add(
            out=r2t[:, :, par : 128 : 2], in0=d2[:], scalar1=e.dc2b[:, 0:1])

    # scoring
    score_ps = e.scp.tile([16, 128], F32, tag="score")   # [(sub,ci), l]
    for sub in range(8):
        sim_ps = e.wps.tile([128, 128], F32, tag="work")
        nc.tensor.matmul(sim_ps[:], e.s1w[:], r2t[:, sub, :],
                         start=True, stop=True)
        sim_sb = e.smp.tile([128, 128], BF16, tag="sim_sb")
        nc.vector.tensor_copy(out=sim_sb[:], in_=sim_ps[:])
        nc.tensor.matmul(
            score_ps[:],
            e.uband[:, (128 - sub) * 2 : (128 - sub) * 2 + 16], sim_sb[:],
            start=(sub == 0), stop=(sub == 7))
    final = e.finp.tile([16, 128], F32, tag="final")
    nc.scalar.activation(out=final[:], in_=score_ps[:], func=ACTF.Sigmoid,
                         bias=e.mlpb[:, 0:1])
    nc.sync.dma_start(
        out=e.io["out"].transpose([1, 0, 2])[sg * 8 : sg * 8 + 8],
        in_=final[:])


def _late_weights(e):
    nc = e.nc
    for t_sb, name in [(e.dc1w, "dc1w"), (e.dc2w, "dc2w"), (e.s1w, "s1w")]:
        nc.sync.dma_start(out=t_sb[:], in_=e.io[name])


def _emit(ctx, tc, io, reps=1):
    e = _setup(ctx, tc, io)
    for _rep in range(reps):
        _phase0(e)
        tmp2_cur = _conv_stage(e, 0, xgs=e.xg0 if _rep == 0 else None)
        if _rep == 0:
            _late_weights(e)
        for sg in range(4):
            qk_sb, tcn = _proj_stage(e, tmp2_cur)
            tmp2_next = _conv_stage(e, sg + 1) if sg < 3 else None
            ar = _attn_stage(e, sg, tmp2_cur, qk_sb, tcn)
            _deconv_score_stage(e, sg, ar)
            tmp2_cur = tmp2_next


# ---------------------------------------------------------------------------
# program build (cached)
# ---------------------------------------------------------------------------

_CACHE = {}


def _build(reps=1):
    key = ("nc", reps)
    if key in _CACHE:
        return _CACHE[key]
    nc = bacc.Bacc("TRN2", target_bir_lowering=False, debug=False)
    d = {}

    def di(name, shape, dt):
        d[name] = nc.dram_tensor(name, shape, dt, kind="ExternalInput").ap()

    di("x", [8, 128, 8, 4, 132], BF16)
    di("c1w", [128, 8, 5, C1], BF16)
    di("c2w", [128, 4, 5, C2], BF16)
    di("dc1w", [128, 8, 4, D1], BF16)
    di("dc2w", [128, 4, 4, D2], BF16)
    di("saq", [128, 2, C2], BF16)
    di("sak", [128, 2, C2], BF16)
    di("cak", [128, 2, C2], BF16)
    di("caq", [128, 3, C2], BF16)
    di("s1w", [128, SIM], BF16)
    di("s2w", [128, 3, SIM], BF16)
    di("vband", [128, 2, 256], BF16)
    di("cavband", [128, 2, 256], BF16)
    di("c1b", [128, 4], F32)
    di("c2b", [128, 2], F32)
    di("dc1b", [128, 4], F32)
    di("dc2b", [128, 1], F32)
    di("sbqk", [128, 2], F32)
    di("cab", [128, 2], F32)
    di("mlpw", [128, 1], F32)
    di("mlpb", [16, 1], F32)
    di("cvec", [128, 3, 2], BF16)
    di("mb", [8, 4, L4], F32)
    d["out"] = nc.dram_tensor("out", [2, SEG, 128], F32,
                              kind="ExternalOutput").ap()
    with tile.TileContext(nc) as tc:
        with ExitStack() as ctx:
            _emit(ctx, tc, d, reps=reps)
    nc.compile()
    _CACHE[key] = nc
    return nc


# ---------------------------------------------------------------------------
# host-side prep (layout/cast only)
# ---------------------------------------------------------------------------

def _chunk_bias(v, nchunk):
    return np.ascontiguousarray(
        np.asarray(v, np.float32).reshape(nchunk, 128).T)


def _band(v):
    # [128, 2, 256] bf16; column 128 of chunk ht = v[ht*128:(ht+1)*128]
    out = np.zeros((128, 2, 256), nbf)
    vv = np.asarray(v, np.float32).reshape(2, 128).T
    out[:, :, 128] = vv.astype(nbf)
    return out


def _wchunks(w, nk):
    # w: [K, ...] -> [128, nk, ...] (zero-pad K up to nk*128)
    w = np.asarray(w, np.float32)
    k = w.shape[0]
    if k < nk * 128:
        w = np.concatenate(
            [w, np.zeros((nk * 128 - k,) + w.shape[1:], np.float32)], 0)
    w = w.reshape((nk, 128) + w.shape[1:])
    perm = (1, 0) + tuple(range(2, w.ndim))
    return np.ascontiguousarray(w.transpose(perm)).astype(nbf)


def prepare_common(inp):
    g = {}
    g["c1w"] = _wchunks(np.asarray(inp["conv1_w"], np.float32)
                        .transpose(1, 2, 0), 8)       # [128,8,5,512]
    g["c2w"] = _wchunks(np.asarray(inp["conv2_w"], np.float32)
                        .transpose(1, 2, 0), 4)       # [128,4,5,256]
    g["dc1w"] = _wchunks(np.asarray(inp["dc1_w"], np.float32)
                         .transpose(0, 2, 1), 8)      # [128,8,4,512]
    g["dc2w"] = _wchunks(np.asarray(inp["dc2_w"], np.float32)
                         .transpose(0, 2, 1), 4)      # [128,4,4,128]
    g["saq"] = _wchunks(inp["sa_wq"], 2)
    g["sak"] = _wchunks(inp["sa_wk"], 2)
    g["cak"] = _wchunks(inp["ca_wk"], 2)
    g["caq"] = _wchunks(inp["ca_wq"], 3)              # [128,3,256]
    g["s1w"] = np.ascontiguousarray(
        np.asarray(inp["sim1_w"], np.float32)).astype(nbf)
    g["s2w"] = _wchunks(inp["sim2_w"], 3)             # [128,3,128]
    g["vband"] = _band(inp["sa_v"])
    g["cavband"] = _band(inp["ca_v"])
    g["c1b"] = _chunk_bias(inp["conv1_b"], 4)
    g["c2b"] = _chunk_bias(inp["conv2_b"], 2)
    g["dc1b"] = _chunk_bias(inp["dc1_b"], 4)
    g["dc2b"] = _chunk_bias(inp["dc2_b"], 1)
    g["sbqk"] = _chunk_bias(
        np.asarray(inp["sa_bq"], np.float32)
        + np.asarray(inp["sa_bk"], np.float32), 2)
    g["cab"] = _chunk_bias(
        np.asarray(inp["ca_bq"], np.float32)
        + np.asarray(inp["ca_bk"], np.float32), 2)
    g["mlpw"] = np.ascontiguousarray(
        np.asarray(inp["mlp_w"], np.float32).reshape(128, 1))
    g["mlpb"] = np.full((16, 1), float(np.asarray(inp["mlp_b"])), np.float32)
    return g


def prepare_core(inp, b):
    o = {}
    x = np.asarray(inp["batch"], np.float32)[b]       # [32,128,1024]
    x = x.transpose(0, 2, 1)                          # [32,1024,128]
    xp = np.zeros((SEG, CIN, 132), np.float32)
    xp[:, :, 2:130] = x
    xp = xp.reshape(8, 4, 8, 128, 132).transpose(0, 3, 2, 1, 4)
    o["x"] = np.ascontiguousarray(xp).astype(nbf)     # [8,128,8,4,132]
    cv = np.zeros((2, 384), np.float32)
    cv[0, :CD] = np.asarray(inp["concept1"], np.float32)[b]
    cv[1, :CD] = np.asarray(inp["concept2"], np.float32)[b]
    o["cvec"] = np.ascontiguousarray(
        cv.reshape(2, 3, 128).transpose(2, 1, 0)).astype(nbf)  # [128,3,2]
    sl = np.asarray(inp["seg_len"], np.int64)[b]      # [32]
    k = np.arange(L4)
    m = np.where(sl[:, None] > 4 * k[None, :], 0.0, NEG).astype(np.float32)
    o["mb"] = np.ascontiguousarray(
        m.reshape(4, 8, L4).transpose(1, 0, 2))       # [8, 4, 32]
    return o


def kernel(**inputs):
    nc = _build()
    common = prepare_common(inputs)
    in_maps = []
    for b in range(B):
        m = dict(common)
        m.update(prepare_core(inputs, b))
        in_maps.append(m)
    res = run_bass_kernel_spmd(nc, in_maps, list(range(B)))
    s1 = np.stack([res.results[b]["out"][0] for b in range(B)])
    s2 = np.stack([res.results[b]["out"][1] for b in range(B)])
    return s1.astype(np.float32), s2.astype(np.float32)
